# revision 29
# baseline (speedup 1.0000x reference)
"""Trainium2 Bass kernel for an AttentionBlock (GroupNorm -> single-head
attention over 64x64 spatial positions -> proj -> residual).

Sharding: 8 zero-communication shards = batch(4) x query-half(2). Each core
receives one batch element's full x[b] [512, 4096], spatially rotated so that
the core's 2048 query positions sit at columns 0:2047. GroupNorm statistics
and attention-over-all-keys are invariant to spatial permutation, so the
rotation is free and every core runs the identical SPMD graph.

Key structural choices:
- GroupNorm is folded into the q/k/v weights: hn = A*x + B per channel, so
  q = (Wq diag(A)) x + (Wq B + qb). The scaled weights are a per-partition
  DVE multiply; the bias corrections are tiny N=1 matmuls. No normalized
  activation tensor is ever materialized - the TensorEngine consumes the
  bf16 copy of x directly.
- Attention is computed transposed: S^T [keys, queries] (stationary = k
  chunk, moving = q block), exp without max subtraction (logits ~N(0,1)),
  softmax denominator accumulated on DVE + one ones-matmul, O^T = v^T @
  expS^T. No transposes of the attention matrix anywhere.
- Softmax normalization is deferred past the output projection (proj is
  linear in O): out = x + pb + (pw @ O_unnorm) * (1/rowsum), so the PE
  never waits on the reciprocal chain. The constant exp shift used to keep
  fp8 expS in range cancels in the same division.
- The S^T and O^T matmuls (72% of PE work) run in fp8-e4m3 DoubleRow
  (2 MACs/cell/cycle); q/k/v/proj matmuls stay bf16.
- x is passed twice from the host: bf16 (stats + matmuls) and f32
  (residual), halving the front-door DMA on the critical path.
"""

import math
import os

import numpy as np

# Problem shapes (hardcoded per the task contract).
B, C, H, W = 4, 512, 64, 64
N = H * W            # 4096 keys per batch element
NQ = N // 2          # 2048 queries per core
P = 128              # SBUF partitions
KC = C // P          # 4 input-channel chunks
DC = C // P          # 4 output-channel chunks
MT = N // P          # 32 key chunks
NQB = 512            # query block (one PSUM bank of f32)
NBLK = NQ // NQB     # 4 query blocks
SEG = 512            # bn_stats segment
NUM_GROUPS = 8
EPS = 1e-5
SCALE = 1.0 / math.sqrt(C)
N_CORES = 8

FP8_S = os.environ.get("ATTN_FP8", "1") == "1"   # S^T in fp8-e4m3 DoubleRow
FP8_O = os.environ.get("ATTN_FP8", "1") == "1"   # O^T in fp8-e4m3 DoubleRow
EXP_BIAS = -2.0 if FP8_O else 0.0   # exp(S*scale + bias); cancels in rowsum
PRE = 3              # next-block S chunks pre-emitted into the epilogue
ROWSUM_PE = False    # softmax denominator on PE (DoubleRow) vs DVE adds

LAST_RESULT = None   # test harness reads the runner result from here


def _build_nc():
    from contextlib import ExitStack

    import concourse.bass as bass
    import concourse.tile as tile
    from concourse import bacc, mybir

    f32 = mybir.dt.float32
    bf16 = mybir.dt.bfloat16
    fp8 = mybir.dt.float8e4
    dt_qk = fp8 if FP8_S else bf16
    dt_v = fp8 if FP8_O else bf16
    Alu = mybir.AluOpType
    Act = mybir.ActivationFunctionType
    DR = mybir.MatmulPerfMode.DoubleRow

    nc = bacc.Bacc(None)

    xb_d = nc.declare_dram_parameter("xb", [C, N], bf16, isOutput=False)
    x_d = nc.declare_dram_parameter("x", [C, N], f32, isOutput=False)
    wq_d = nc.declare_dram_parameter("wqT", [C, C], bf16, isOutput=False)
    wk_d = nc.declare_dram_parameter("wkT", [C, C], bf16, isOutput=False)
    wv_d = nc.declare_dram_parameter("wvT", [C, C], bf16, isOutput=False)
    wp_d = nc.declare_dram_parameter("wpT", [C, C], bf16, isOutput=False)
    qb_d = nc.declare_dram_parameter("qb", [C, 1], f32, isOutput=False)
    kb_d = nc.declare_dram_parameter("kb", [C, 1], f32, isOutput=False)
    pb_d = nc.declare_dram_parameter("pb", [C, 1], f32, isOutput=False)
    vb_d = nc.declare_dram_parameter("vb_row", [1, C], f32, isOutput=False)
    gnm_d = nc.declare_dram_parameter("gn_mask", [C, NUM_GROUPS], f32, isOutput=False)
    gnb_d = nc.declare_dram_parameter("gn_bcast", [NUM_GROUPS, C], f32, isOutput=False)
    beta_d = nc.declare_dram_parameter("gn_beta", [C, 1], f32, isOutput=False)
    out_d = nc.declare_dram_parameter("out", [C, NQ], f32, isOutput=True)

    xbr = xb_d[:, :].rearrange("(kc p) n -> p kc n", p=P)      # [128, 4, 4096]
    xr = x_d[:, :].rearrange("(kc p) n -> p kc n", p=P)        # [128, 4, 4096]
    outr = out_d[:, :].rearrange("(ec p) n -> p ec n", p=P)    # [128, 4, 2048]

    with tile.TileContext(nc) as tc:
        with ExitStack() as ctx:
            consts = ctx.enter_context(tc.tile_pool(name="consts", bufs=1))
            wsp = ctx.enter_context(tc.tile_pool(name="wsp", bufs=1))
            kpool = ctx.enter_context(tc.tile_pool(name="kpool", bufs=1))
            qpool = ctx.enter_context(tc.tile_pool(name="qpool", bufs=1))
            vpool = ctx.enter_context(tc.tile_pool(name="vpool", bufs=1))

            # ---- front-door DMAs; xb chunks first (stats critical path) ----
            with tc.tile_pool(name="xbp", bufs=1) as xbp:
                xb_t = xbp.tile([P, KC, N], bf16, tag="xb")
                for kc in range(KC):
                    nc.sync.dma_start(out=xb_t[:, kc, :], in_=xbr[:, kc, :])

                wq_t = consts.tile([P, KC, C], bf16, tag="wq")
                nc.sync.dma_start(out=wq_t[:], in_=wq_d[:, :].rearrange("(kc p) d -> p kc d", p=P))
                wk_t = consts.tile([P, KC, C], bf16, tag="wk")
                nc.sync.dma_start(out=wk_t[:], in_=wk_d[:, :].rearrange("(kc p) d -> p kc d", p=P))
                wv_t = consts.tile([P, KC, C], bf16, tag="wv")
                nc.sync.dma_start(out=wv_t[:], in_=wv_d[:, :].rearrange("(kc p) d -> p kc d", p=P))
                wp_t = consts.tile([P, KC, C], bf16, tag="wp")
                nc.sync.dma_start(out=wp_t[:], in_=wp_d[:, :].rearrange("(kc p) d -> p kc d", p=P))

                qb_t = consts.tile([P, DC, 1], f32, tag="qb")
                nc.sync.dma_start(out=qb_t[:], in_=qb_d[:, :].rearrange("(dc p) o -> p dc o", p=P))
                kb_t = consts.tile([P, DC, 1], f32, tag="kb")
                nc.sync.dma_start(out=kb_t[:], in_=kb_d[:, :].rearrange("(dc p) o -> p dc o", p=P))
                pb_t = consts.tile([P, DC, 1], f32, tag="pb")
                nc.sync.dma_start(out=pb_t[:], in_=pb_d[:, :].rearrange("(dc p) o -> p dc o", p=P))
                vbrow_t = consts.tile([1, C], f32, tag="vbrow")
                nc.sync.dma_start(out=vbrow_t[:], in_=vb_d[:, :])
                gnm_t = consts.tile([P, KC, NUM_GROUPS], f32, tag="gnm")
                nc.sync.dma_start(out=gnm_t[:], in_=gnm_d[:, :].rearrange("(kc p) g -> p kc g", p=P))
                gnb_t = consts.tile([NUM_GROUPS, C], f32, tag="gnb")
                nc.sync.dma_start(out=gnb_t[:], in_=gnb_d[:, :])
                beta_t = consts.tile([P, KC, 1], f32, tag="beta")
                nc.sync.dma_start(out=beta_t[:], in_=beta_d[:, :].rearrange("(kc p) o -> p kc o", p=P))

                ones_t = consts.tile([P, 1], f32, tag="ones")
                nc.vector.memset(ones_t[:], 1.0)
                ones1_t = consts.tile([1, P], f32, tag="ones1")
                nc.vector.memset(ones1_t[:], 1.0)
                ebias_t = consts.tile([P, 1], f32, tag="ebias")
                nc.vector.memset(ebias_t[:], EXP_BIAS)
                # fp8/bf16 ones column(s): stationary operand of the
                # rowsum-over-partitions matmul (dtype must match expS).
                # The fp8 DoubleRow interleave step must be 16-byte aligned,
                # so the pair axis is padded to 16 columns.
                dt_p_c = fp8 if FP8_O else bf16
                if FP8_O:
                    onesp_full = consts.tile([P, 2, 16], dt_p_c, tag="onesp")
                    nc.vector.memset(onesp_full[:], 1.0)
                    onesp_t = onesp_full[:, :, 0:1]
                else:
                    onesp_full = consts.tile([P, 1], dt_p_c, tag="onesp")
                    nc.vector.memset(onesp_full[:], 1.0)
                    onesp_t = onesp_full[:]

                # per-channel GroupNorm scale/shift [A, B], filled below
                ab_t = consts.tile([P, KC, 2], f32, tag="ab")
                bb_t = consts.tile([P, KC, 1], bf16, tag="bb")
                qbe_t = consts.tile([P, DC, 1], f32, tag="qbe")
                kbe_t = consts.tile([P, DC, 1], f32, tag="kbe")
                vb_t = consts.tile([P, C], f32, tag="vb")

                k_t = kpool.tile([P, DC, N], dt_qk, tag="k")
                q_t = qpool.tile([P, DC, NQ], dt_qk, tag="q")
                vT_t = vpool.tile([P, MT, C], dt_v, tag="vT")
                wqs_t = wsp.tile([P, KC, C], bf16, tag="wqs")
                wks_t = wsp.tile([P, KC, C], bf16, tag="wks")
                wvs_t = wsp.tile([P, KC, C], bf16, tag="wvs")

                # ---------------- GroupNorm statistics ----------------
                gn_scope = ExitStack()
                gnw = gn_scope.enter_context(tc.tile_pool(name="gnw", bufs=2))
                gnps = gn_scope.enter_context(
                    tc.tile_pool(name="gnps", bufs=1, space="PSUM"))
                gstats = gnps.tile([NUM_GROUPS, 2], f32, tag="gps")
                for kc in range(KC):
                    stats = gnw.tile([P, N // SEG, 6], f32, tag="stats")
                    for s in range(N // SEG):
                        nc.vector.bn_stats(out=stats[:, s, :],
                                           in_=xb_t[:, kc, s * SEG:(s + 1) * SEG])
                    mv = gnw.tile([P, 2], f32, tag="mv")
                    nc.vector.bn_aggr(out=mv[:], in_=stats[:])
                    # per-(partition,chunk) [mean, E[x^2]]
                    s1s2 = gnw.tile([P, 2], f32, tag="s1s2")
                    nc.vector.tensor_copy(out=s1s2[:, 0:1], in_=mv[:, 0:1])
                    sq = gnw.tile([P, 1], f32, tag="sq")
                    nc.vector.tensor_mul(out=sq[:], in0=mv[:, 0:1], in1=mv[:, 0:1])
                    nc.vector.tensor_add(out=s1s2[:, 1:2], in0=mv[:, 1:2], in1=sq[:])
                    nc.tensor.matmul(out=gstats[:], lhsT=gnm_t[:, kc, :], rhs=s1s2[:],
                                     start=(kc == 0), stop=(kc == KC - 1))

                # group scalars: vals = [rstd, -mean*rstd] on 8 partitions.
                # gstats[g] = sum over the group's 64 partitions of
                # [mean_p, E2_p], each over that partition's 4096 columns.
                gm = gnw.tile([NUM_GROUPS, 4], f32, tag="gm")
                vals = gnw.tile([NUM_GROUPS, 2], f32, tag="vals")
                eps_t = gnw.tile([NUM_GROUPS, 1], f32, tag="eps")
                nc.vector.memset(eps_t[:], EPS)
                nc.vector.tensor_scalar_mul(out=gm[:, 0:1], in0=gstats[:, 0:1], scalar1=1.0 / 64.0)
                nc.vector.tensor_scalar_mul(out=gm[:, 1:2], in0=gstats[:, 1:2], scalar1=1.0 / 64.0)
                nc.vector.tensor_mul(out=gm[:, 2:3], in0=gm[:, 0:1], in1=gm[:, 0:1])
                nc.vector.tensor_sub(out=gm[:, 3:4], in0=gm[:, 1:2], in1=gm[:, 2:3])
                nc.scalar.activation(out=gm[:, 3:4], in_=gm[:, 3:4], func=Act.Sqrt, bias=eps_t[:])
                nc.vector.reciprocal(out=vals[:, 0:1], in_=gm[:, 3:4])
                nc.vector.tensor_mul(out=vals[:, 1:2], in0=gm[:, 0:1], in1=vals[:, 0:1])
                nc.vector.tensor_scalar_mul(out=vals[:, 1:2], in0=vals[:, 1:2], scalar1=-1.0)

                # broadcast to channels: A = gamma*rstd, B = beta - gamma*mean*rstd
                for kc in range(KC):
                    abp = gnps.tile([P, 2], f32, tag="abp")
                    nc.tensor.matmul(out=abp[:], lhsT=gnb_t[:, kc * P:(kc + 1) * P],
                                     rhs=vals[:], start=True, stop=True)
                    nc.vector.tensor_copy(out=ab_t[:, kc, 0:1], in_=abp[:, 0:1])
                    nc.vector.tensor_add(out=ab_t[:, kc, 1:2], in0=abp[:, 1:2], in1=beta_t[:, kc, :])
                    nc.scalar.activation(out=bb_t[:, kc, :], in_=ab_t[:, kc, 1:2], func=Act.Copy)

                # ---- fold GN into weights: Ws = diag(A) @ W^T (per-partition) ----
                for kc in range(KC):
                    nc.vector.tensor_scalar_mul(out=wqs_t[:, kc, :], in0=wq_t[:, kc, :],
                                                scalar1=ab_t[:, kc, 0:1])
                    nc.vector.tensor_scalar_mul(out=wks_t[:, kc, :], in0=wk_t[:, kc, :],
                                                scalar1=ab_t[:, kc, 0:1])
                    nc.vector.tensor_scalar_mul(out=wvs_t[:, kc, :], in0=wv_t[:, kc, :],
                                                scalar1=ab_t[:, kc, 0:1])

                # effective biases: qbe = qb + Wq @ B, kbe = kb + Wk @ B
                for dc in range(DC):
                    bq = gnps.tile([P, 1], f32, tag="bqk", name="bq", bufs=2)
                    for kc in range(KC):
                        nc.tensor.matmul(out=bq[:], lhsT=wq_t[:, kc, dc * P:(dc + 1) * P],
                                         rhs=bb_t[:, kc, :], start=(kc == 0), stop=(kc == KC - 1))
                    nc.vector.tensor_add(out=qbe_t[:, dc, :], in0=bq[:], in1=qb_t[:, dc, :])
                    bk = gnps.tile([P, 1], f32, tag="bqk", name="bk", bufs=2)
                    for kc in range(KC):
                        nc.tensor.matmul(out=bk[:], lhsT=wk_t[:, kc, dc * P:(dc + 1) * P],
                                         rhs=bb_t[:, kc, :], start=(kc == 0), stop=(kc == KC - 1))
                    nc.vector.tensor_add(out=kbe_t[:, dc, :], in0=bk[:], in1=kb_t[:, dc, :])

                # v bias row -> broadcast to 128 partitions via K=1 matmul
                bv = gnps.tile([1, C], f32, tag="bv")
                for kc in range(KC):
                    nc.tensor.matmul(out=bv[:], lhsT=bb_t[:, kc, :], rhs=wv_t[:, kc, :],
                                     start=(kc == 0), stop=(kc == KC - 1))
                vrow = gnw.tile([1, C], f32, tag="vrow")
                nc.vector.tensor_add(out=vrow[:], in0=bv[:], in1=vbrow_t[:])
                vbc = gnps.tile([P, C], f32, tag="vbc")
                nc.tensor.matmul(out=vbc[:], lhsT=ones1_t[:], rhs=vrow[:], start=True, stop=True)
                nc.scalar.activation(out=vb_t[:], in_=vbc[:], func=Act.Copy)
                gn_scope.close()

                # ---------------- q / k / vT ----------------
                with tc.tile_pool(name="qkvps", bufs=4, space="PSUM") as qkvps:
                    for dc in range(DC):
                        for nb in range(NQ // NQB):
                            ps = qkvps.tile([P, NQB], f32, tag="mm")
                            for kc in range(KC):
                                nc.tensor.matmul(out=ps[:], lhsT=wqs_t[:, kc, dc * P:(dc + 1) * P],
                                                 rhs=xb_t[:, kc, nb * NQB:(nb + 1) * NQB],
                                                 start=(kc == 0), stop=(kc == KC - 1))
                            nc.scalar.activation(out=q_t[:, dc, nb * NQB:(nb + 1) * NQB], in_=ps[:],
                                                 func=Act.Identity, bias=qbe_t[:, dc, :])
                        for mb in range(N // NQB):
                            ps = qkvps.tile([P, NQB], f32, tag="mm")
                            for kc in range(KC):
                                nc.tensor.matmul(out=ps[:], lhsT=wks_t[:, kc, dc * P:(dc + 1) * P],
                                                 rhs=xb_t[:, kc, mb * NQB:(mb + 1) * NQB],
                                                 start=(kc == 0), stop=(kc == KC - 1))
                            nc.scalar.activation(out=k_t[:, dc, mb * NQB:(mb + 1) * NQB], in_=ps[:],
                                                 func=Act.Identity, bias=kbe_t[:, dc, :])
                    for mi in range(MT):
                        ps = qkvps.tile([P, NQB], f32, tag="mm")
                        for kc in range(KC):
                            nc.tensor.matmul(out=ps[:], lhsT=xb_t[:, kc, mi * P:(mi + 1) * P],
                                             rhs=wvs_t[:, kc, :],
                                             start=(kc == 0), stop=(kc == KC - 1))
                        nc.vector.tensor_add(out=vT_t[:, mi, :], in0=ps[:], in1=vb_t[:])

            # ---------------- attention + proj + residual ----------------
            OCH = 2 if FP8_O else 1        # m-chunks per O accumulation unit
            OU = MT // OCH                 # O units per block
            dt_p = fp8 if FP8_O else bf16  # expS dtype

            with tc.tile_pool(name="aw", bufs=6) as aw, \
                 tc.tile_pool(name="accp", bufs=2) as accp, \
                 tc.tile_pool(name="onp", bufs=2) as onp, \
                 tc.tile_pool(name="resp", bufs=4) as resp, \
                 tc.tile_pool(name="ps_s", bufs=2, space="PSUM") as ps_s, \
                 tc.tile_pool(name="ps_o", bufs=4, space="PSUM") as ps_o, \
                 tc.tile_pool(name="ps_r", bufs=2, space="PSUM") as ps_r:

                def make_state(blk):
                    nq0 = blk * NQB
                    if ROWSUM_PE:
                        rs = ps_r.tile([1, NQB], f32, tag="r", name=f"rs{blk}")
                    else:
                        rs = accp.tile([P, NQB], f32, tag="accum", name=f"acc{blk}")
                    opsums = [ps_o.tile([P, NQB], f32, tag="o", name=f"ops{blk}_{d}")
                              for d in range(DC)]
                    xres = resp.tile([P, DC, NQB], f32, tag="xres", name=f"xres{blk}")
                    nc.sync.dma_start(out=xres[:], in_=xr[:, :, nq0:nq0 + NQB])
                    for ec in range(DC):
                        nc.vector.tensor_scalar_add(out=xres[:, ec, :], in0=xres[:, ec, :],
                                                    scalar1=pb_t[:, ec, :])
                    return dict(nq0=nq0, rs=rs, opsums=opsums, xres=xres,
                                es={}, next_s=0, next_o=0)

                def emit_S(st, mi):
                    nq0 = st["nq0"]
                    sps = ps_s.tile([P, NQB], f32, tag="s", name="sps")
                    if FP8_S:
                        for j in range(2):
                            nc.tensor.matmul(out=sps[:],
                                             lhsT=k_t[:, 2 * j:2 * j + 2, mi * P:(mi + 1) * P],
                                             rhs=q_t[:, 2 * j:2 * j + 2, nq0:nq0 + NQB],
                                             start=(j == 0), stop=(j == 1),
                                             perf_mode=DR)
                    else:
                        for dc2 in range(DC):
                            nc.tensor.matmul(out=sps[:],
                                             lhsT=k_t[:, dc2, mi * P:(mi + 1) * P],
                                             rhs=q_t[:, dc2, nq0:nq0 + NQB],
                                             start=(dc2 == 0), stop=(dc2 == DC - 1))
                    if FP8_O:
                        if mi % 2 == 0:
                            st["es"][mi // 2] = aw.tile([P, 2, NQB], dt_p, tag="exps", name="es")
                        es = st["es"][mi // 2]
                        eslot = es[:, mi % 2, :]
                    else:
                        es = st["es"][mi] = aw.tile([P, NQB], dt_p, tag="exps", name="es")
                        eslot = es[:]
                    nc.scalar.activation(out=eslot, in_=sps[:], func=Act.Exp,
                                         scale=SCALE, bias=ebias_t[:])
                    if not ROWSUM_PE:
                        if mi == 0:
                            nc.vector.tensor_copy(out=st["rs"][:], in_=eslot)
                        else:
                            nc.vector.tensor_add(out=st["rs"][:], in0=st["rs"][:], in1=eslot)

                def emit_O(st, u):
                    es = st["es"].pop(u)
                    for dc2 in range(DC):
                        if FP8_O:
                            nc.tensor.matmul(out=st["opsums"][dc2][:],
                                             lhsT=vT_t[:, 2 * u:2 * u + 2, dc2 * P:(dc2 + 1) * P],
                                             rhs=es[:],
                                             start=(u == 0), stop=(u == OU - 1),
                                             perf_mode=DR)
                        else:
                            nc.tensor.matmul(out=st["opsums"][dc2][:],
                                             lhsT=vT_t[:, u, dc2 * P:(dc2 + 1) * P],
                                             rhs=es[:],
                                             start=(u == 0), stop=(u == OU - 1))
                    if ROWSUM_PE:
                        # softmax denominator: rowsum over the key partitions,
                        # accumulated on the PE alongside the O matmuls
                        nc.tensor.matmul(out=st["rs"][:], lhsT=onesp_t[:], rhs=es[:],
                                         start=(u == 0), stop=(u == OU - 1),
                                         perf_mode=DR if FP8_O else None)

                states = {0: None}
                states[0] = make_state(0)
                for blk in range(NBLK):
                    st = states[blk]
                    nq0 = st["nq0"]
                    while st["next_s"] < MT:
                        emit_S(st, st["next_s"])
                        st["next_s"] += 1
                        done_u = st["next_s"] // OCH
                        if st["next_s"] % OCH == 0 and st["next_o"] < done_u - 1:
                            emit_O(st, st["next_o"])
                            st["next_o"] += 1
                    while st["next_o"] < OU:
                        emit_O(st, st["next_o"])
                        st["next_o"] += 1

                    # reciprocal of the softmax denominator (normalization
                    # is applied after proj, so none of this gates the PE)
                    if ROWSUM_PE:
                        rsrow = st["rs"]
                    else:
                        rsrow = ps_r.tile([1, NQB], f32, tag="r", name="rsum")
                        nc.tensor.matmul(out=rsrow[:], lhsT=ones_t[:], rhs=st["rs"][:],
                                         start=True, stop=True)
                    rrec = accp.tile([1, NQB], f32, tag="rrec")
                    nc.vector.reciprocal(out=rrec[:], in_=rsrow[:])

                    # unnormalized O^T -> bf16 (ACT, independent of rowsum)
                    onrm = onp.tile([P, DC, NQB], bf16, tag="on")
                    for dc2 in range(DC):
                        nc.scalar.activation(out=onrm[:, dc2, :], in_=st["opsums"][dc2][:],
                                             func=Act.Copy)

                    # keep the PE warm through the epilogue with the next
                    # block's first S chunks
                    if blk + 1 < NBLK:
                        nst = states[blk + 1] = make_state(blk + 1)
                        for _ in range(PRE):
                            emit_S(nst, nst["next_s"])
                            nst["next_s"] += 1

                    rbc = ps_r.tile([P, NQB], f32, tag="r", name="rbc")
                    nc.tensor.matmul(out=rbc[:], lhsT=ones1_t[:], rhs=rrec[:],
                                     start=True, stop=True)
                    rbs = accp.tile([P, NQB], f32, tag="rbs")
                    nc.scalar.activation(out=rbs[:], in_=rbc[:], func=Act.Copy)

                    # proj on unnormalized O, then scale + residual on DVE
                    xres = st["xres"]
                    for ec in range(DC):
                        yps = ps_s.tile([P, NQB], f32, tag="s", name="yps")
                        for dc2 in range(DC):
                            nc.tensor.matmul(out=yps[:],
                                             lhsT=wp_t[:, dc2, ec * P:(ec + 1) * P],
                                             rhs=onrm[:, dc2, :],
                                             start=(dc2 == 0), stop=(dc2 == DC - 1))
                        ytmp = accp.tile([P, NQB], f32, tag="ytmp")
                        nc.vector.tensor_mul(out=ytmp[:], in0=yps[:], in1=rbs[:])
                        nc.vector.tensor_add(out=xres[:, ec, :], in0=xres[:, ec, :], in1=ytmp[:])
                        nc.sync.dma_start(out=outr[:, ec, nq0:nq0 + NQB], in_=xres[:, ec, :])

    if not nc.is_finalized():
        nc.finalize()
    return nc


def _host_inputs(x, gn_gamma, gn_beta, qw, qb, kw, kb, vw, vb, pw, pb):
    import ml_dtypes

    bf = ml_dtypes.bfloat16
    f32 = np.float32

    wqT = np.ascontiguousarray(qw.T).astype(bf)
    wkT = np.ascontiguousarray(kw.T).astype(bf)
    wvT = np.ascontiguousarray(vw.T).astype(bf)
    wpT = np.ascontiguousarray(pw.T).astype(bf)
    qb_ = np.ascontiguousarray(qb.reshape(C, 1)).astype(f32)
    kb_ = np.ascontiguousarray(kb.reshape(C, 1)).astype(f32)
    pb_ = np.ascontiguousarray(pb.reshape(C, 1)).astype(f32)
    vb_row = np.ascontiguousarray(vb.reshape(1, C)).astype(f32)

    groups = np.arange(C) // (C // NUM_GROUPS)
    gn_mask = np.zeros((C, NUM_GROUPS), f32)
    gn_mask[np.arange(C), groups] = 1.0
    gn_bcast = np.zeros((NUM_GROUPS, C), f32)
    gn_bcast[groups, np.arange(C)] = np.asarray(gn_gamma, f32)
    beta_ = np.ascontiguousarray(np.asarray(gn_beta, f32).reshape(C, 1))

    shared = {
        "wqT": wqT, "wkT": wkT, "wvT": wvT, "wpT": wpT,
        "qb": qb_, "kb": kb_, "pb": pb_, "vb_row": vb_row,
        "gn_mask": gn_mask, "gn_bcast": gn_bcast, "gn_beta": beta_,
    }

    xf = np.asarray(x, f32).reshape(B, C, N)
    in_maps = []
    for core in range(N_CORES):
        b, h = divmod(core, 2)
        xc = np.ascontiguousarray(np.roll(xf[b], -h * NQ, axis=1))
        m = dict(shared)
        m["x"] = xc
        m["xb"] = xc.astype(bf)
        in_maps.append(m)
    return in_maps


def kernel(x, gn_gamma, gn_beta, qw, qb, kw, kb, vw, vb, pw, pb):
    global LAST_RESULT
    # The NTFF trace path needs hooks this environment doesn't ship; make
    # sure a stray BASS_TRACE can't route us into it.
    os.environ["BASS_NEVER_TRACE"] = "1"
    from concourse.bass_utils import run_bass_kernel_spmd

    x = np.asarray(x)
    in_maps = _host_inputs(x, np.asarray(gn_gamma), np.asarray(gn_beta),
                           np.asarray(qw), np.asarray(qb), np.asarray(kw),
                           np.asarray(kb), np.asarray(vw), np.asarray(vb),
                           np.asarray(pw), np.asarray(pb))

    nc = _build_nc()
    res = run_bass_kernel_spmd(nc, in_maps, core_ids=list(range(N_CORES)))
    LAST_RESULT = res

    out = np.empty((B, C, N), np.float32)
    for core in range(N_CORES):
        b, h = divmod(core, 2)
        out[b][:, h * NQ:(h + 1) * NQ] = np.asarray(res.results[core]["out"])
    return out.reshape(B, C, H, W)
